# revision 1
# baseline (speedup 1.0000x reference)
"""Trainium2 Bass kernel: 4-layer sliding-window causal transformer (B=2, S=2048,
D=768, H=12, FF=3072, window=128) on 8 NeuronCores.

Sharding: 2 batch groups x 4-way sequence split. Core c handles batch b=c//4,
owning 512 tokens: A=[256q,256q+256) and B=[1024+256q,1024+256q+256) (q=c%4).
The B half covers all possibly-invalid rows (lengths >= S/2), so the
reference's dense-attention behaviour for rows >= length is computed uniformly
(256 dense rows per core) and merged by a select mask. Per layer, one
AllGather (bf16 K/V) within each 4-core group provides remote keys/values;
the two core-dependent halo key blocks are fetched with register-offset DMAs
whose offsets arrive as per-core input data (keeps the SPMD program uniform).

Everything on-chip is feature-major ([d on partitions, tokens free]) so no
transposes are needed anywhere; matmuls run in bf16 with f32 accumulation;
LayerNorm/softmax math in f32.
"""
import contextlib
import numpy as np
import ml_dtypes

import concourse.bass as bass
import concourse.tile as tile
from concourse import mybir
from concourse.vector_clock import ScopedClock
from concourse.tile_rust import add_dep_helper
from concourse.bass_utils import run_bass_kernel_spmd

F32 = mybir.dt.float32
BF16 = mybir.dt.bfloat16
U8 = mybir.dt.uint8
I32 = mybir.dt.int32
AF = mybir.ActivationFunctionType

B, S, D, H, L, FF, WINDOW, HD = 2, 2048, 768, 12, 4, 3072, 128, 64
C = D // 128          # 6 feature chunks
T = 512               # tokens per core
G = 16                # 128-token blocks per batch
FFC = FF // 128       # 24
AGW = D * T           # 393216 elements in each AG section (kT / V)
GROUPS = [[0, 1, 2, 3], [4, 5, 6, 7]]
N_CORES = 8
EPS = 1e-6
SCALE = 1.0 / np.sqrt(HD)


def _owner(g):
    return g // 2 if g < 8 else (g - 8) // 2


def _lb(g):
    return g % 2 if g < 8 else 2 + (g - 8) % 2


def _rows_for(q):
    return list(range(256 * q, 256 * q + 256)) + list(range(1024 + 256 * q, 1024 + 256 * q + 256))


class TC(tile.TileContext):
    """TileContext workarounds for this toolchain's walrus sync-wait limit:
    at most ONE sync wait per instruction; excess waits move onto injected
    no-fuse NoOps running just before it on the same engine."""
    MAX_WAITS = 1

    def _split_waits(self, insts):
        out = []
        for inst in insts:
            si = getattr(inst, "sync_info", None)
            waits = list(si.on_wait) if si is not None else []
            if len(waits) > self.MAX_WAITS and inst.engine != mybir.EngineType.Unassigned:
                keep = waits[-self.MAX_WAITS:]
                moved = waits[:-self.MAX_WAITS]
                inst.sync_info = mybir.SyncInfo(on_wait=keep, on_update=list(si.on_update))
                for i in range(0, len(moved), self.MAX_WAITS):
                    out.append(mybir.InstNoOp(
                        name=f"{inst.name}-waitnop-{i}",
                        engine=inst.engine,
                        sync_info=mybir.SyncInfo(
                            on_wait=moved[i:i + self.MAX_WAITS], on_update=[]),
                        bass_nofuse=True,
                    ))
            out.append(inst)
        return out

    def _lower_ordered_insts(self, ordered):
        for bb_name in list(ordered.keys()):
            ordered[bb_name] = self._split_waits(ordered[bb_name])
        return super()._lower_ordered_insts(ordered)

    def _drain_and_barrier(self, tick_clock, wait_clock):
        drain_inst = self.nc.sync.drain()
        wait_clock.add_sem_waits(drain_inst.ins, ScopedClock({None: tick_clock.global_clock}))
        si = drain_inst.ins.sync_info
        if si is not None and len(si.on_wait) > self.MAX_WAITS:
            waits = list(si.on_wait)
            drain_inst.ins.sync_info = mybir.SyncInfo(
                on_wait=waits[: self.MAX_WAITS], on_update=list(si.on_update))
            rest = waits[self.MAX_WAITS:]
            for i in range(0, len(rest), self.MAX_WAITS):
                extra = self.nc.sync.drain()
                extra.ins.sync_info = mybir.SyncInfo(
                    on_wait=rest[i: i + self.MAX_WAITS], on_update=[])
        self.nc.all_engine_barrier()
        assert self.sems is not None
        popped = self.nc._tile_sem_poison_stack.pop()
        assert popped is self._sem_poison
        self.nc.clear_and_free_semaphores(list(self.sems.allocated().values()))
        self.nc.all_engine_barrier()


def build_nc(n_layers=L, no_collective=False):
    nc = bass.Bass(num_devices=N_CORES)

    # ---------------- I/O ----------------
    xT = nc.dram_tensor("xT", [128, C, T], F32, kind="ExternalInput")
    posT = nc.dram_tensor("posT", [128, C, T], F32, kind="ExternalInput")
    tokf = nc.dram_tensor("tokf", [128, C], F32, kind="ExternalInput")
    eg = nc.dram_tensor("eg", [128, C], F32, kind="ExternalInput")
    ebi = nc.dram_tensor("ebi", [128, C], F32, kind="ExternalInput")
    lng = nc.dram_tensor("lng", [128, L, 2, C], F32, kind="ExternalInput")
    lnb = nc.dram_tensor("lnb", [128, L, 2, C], F32, kind="ExternalInput")
    qbi = nc.dram_tensor("qbi", [128, L, C], F32, kind="ExternalInput")
    obi = nc.dram_tensor("obi", [128, L, C], F32, kind="ExternalInput")
    f1bi = nc.dram_tensor("f1bi", [128, L, FFC], F32, kind="ExternalInput")
    f2bi = nc.dram_tensor("f2bi", [128, L, C], F32, kind="ExternalInput")
    mbandi = nc.dram_tensor("mbandi", [128, 1024], BF16, kind="ExternalInput")
    mseli = nc.dram_tensor("mseli", [128, 256], BF16, kind="ExternalInput")
    hoffi = nc.dram_tensor("hoffi", [1, 4], I32, kind="ExternalInput")
    wq = nc.dram_tensor("wq", [L, D, D], BF16, kind="ExternalInput")
    wk = nc.dram_tensor("wk", [L, D, D], BF16, kind="ExternalInput")
    wv = nc.dram_tensor("wv", [L, D, D], BF16, kind="ExternalInput")
    wo = nc.dram_tensor("wo", [L, D, D], BF16, kind="ExternalInput")
    w1 = nc.dram_tensor("w1", [L, D, FF], BF16, kind="ExternalInput")
    w2 = nc.dram_tensor("w2", [L, FF, D], BF16, kind="ExternalInput")
    out_d = nc.dram_tensor("out", [128, C, T], F32, kind="ExternalOutput")

    agis = [nc.dram_tensor(f"agi{l}", [2 * AGW], BF16) for l in range(n_layers)]
    agos = [nc.dram_tensor(f"ago{l}", [4, 2 * AGW], BF16) for l in range(n_layers)]

    ln_ctr = [0]

    with contextlib.ExitStack() as ctx:
        tc = ctx.enter_context(TC(nc))
        # ---------------- persistent pools ----------------
        cst = ctx.enter_context(tc.tile_pool(name="cst", bufs=1))
        hpool = ctx.enter_context(tc.tile_pool(name="hp", bufs=1))
        big = ctx.enter_context(tc.tile_pool(name="big", bufs=1))
        sgl = ctx.enter_context(tc.tile_pool(name="sgl", bufs=1))
        scr4 = ctx.enter_context(tc.tile_pool(name="scr4", bufs=4))
        scr2 = ctx.enter_context(tc.tile_pool(name="scr2", bufs=2))
        scr3 = ctx.enter_context(tc.tile_pool(name="scr3", bufs=3))
        wp4 = ctx.enter_context(tc.tile_pool(name="wp4", bufs=8))
        wp1 = ctx.enter_context(tc.tile_pool(name="wp1", bufs=6))
        wp2 = ctx.enter_context(tc.tile_pool(name="wp2", bufs=4))
        epB = ctx.enter_context(tc.tile_pool(name="epB", bufs=2))
        epD = ctx.enter_context(tc.tile_pool(name="epD", bufs=4))

        regKA = ctx.enter_context(nc.gpsimd.register("regKA"))
        regKB = ctx.enter_context(nc.gpsimd.register("regKB"))
        regVA = ctx.enter_context(nc.gpsimd.register("regVA"))
        regVB = ctx.enter_context(nc.gpsimd.register("regVB"))

        # ---------------- load constants ----------------
        def cload(nm, shape, dt, src):
            t = cst.tile(shape, dt, tag=nm, name=nm)
            nc.sync.dma_start(out=t, in_=src)
            return t

        tok_s = cload("tok_s", [128, C], F32, tokf[:, :])
        eg_s = cload("eg_s", [128, C], F32, eg[:, :])
        eb_s = cload("eb_s", [128, C], F32, ebi[:, :])
        lng_s = cload("lng_s", [128, L, 2, C], F32, lng[:, :, :, :])
        lnb_s = cload("lnb_s", [128, L, 2, C], F32, lnb[:, :, :, :])
        qb_s = cload("qb_s", [128, L, C], F32, qbi[:, :, :])
        ob_s = cload("ob_s", [128, L, C], F32, obi[:, :, :])
        f1b_s = cload("f1b_s", [128, L, FFC], F32, f1bi[:, :, :])
        f2b_s = cload("f2b_s", [128, L, C], F32, f2bi[:, :, :])
        mband = cload("mband", [128, 1024], BF16, mbandi[:, :])
        msel = cload("msel", [128, 256], BF16, mseli[:, :])
        hoff_s = cload("hoff_s", [1, 4], I32, hoffi[:, :])

        onesP = cst.tile([128, 1], F32)
        nc.vector.memset(onesP, 1.0)
        onesPb = cst.tile([128, 1], BF16)
        nc.vector.memset(onesPb, 1.0)
        onesPP = cst.tile([128, 128], F32)
        nc.vector.memset(onesPP, 1.0)
        eps_row = cst.tile([1, 1], F32)
        nc.vector.memset(eps_row, EPS)

        nc.gpsimd.reg_load(regKA, hoff_s[0:1, 0:1])
        nc.gpsimd.reg_load(regKB, hoff_s[0:1, 1:2])
        nc.gpsimd.reg_load(regVA, hoff_s[0:1, 2:3])
        nc.gpsimd.reg_load(regVB, hoff_s[0:1, 3:4])

        # ---------------- h / embeddings ----------------
        h = hpool.tile([128, C, T], F32)
        nc.sync.dma_start(out=h, in_=xT[:, :, :])
        for k in range(C):
            pk = scr2.tile([128, T], F32, tag="lnt")
            nc.sync.dma_start(out=pk, in_=posT[:, k, :])
            nc.vector.tensor_add(out=h[:, k, :], in0=h[:, k, :], in1=pk)
            nc.vector.tensor_scalar_add(h[:, k, :], h[:, k, :], tok_s[:, k:k + 1])

        def emit_ln(g_ap, b_ap):
            """post-LN over features (partition axis), h updated in place."""
            ln_ctr[0] += 1
            with tc.tile_pool(name=f"lnps{ln_ctr[0]}", bufs=1, space="PSUM") as lnps, \
                 tc.tile_pool(name=f"lnps2_{ln_ctr[0]}", bufs=2, space="PSUM") as lnps2:
                s1 = lnps.tile([1, T], F32, tag="s1")
                s2 = lnps.tile([1, T], F32, tag="s2")
                for k in range(C):
                    sq = scr2.tile([128, T], BF16, tag="sq")
                    nc.vector.tensor_mul(out=sq, in0=h[:, k, :], in1=h[:, k, :])
                    nc.tensor.matmul(s1, onesP[:, :], h[:, k, :],
                                     start=(k == 0), stop=(k == C - 1))
                    nc.tensor.matmul(s2, onesPb[:, :], sq,
                                     start=(k == 0), stop=(k == C - 1))
                mean = scr4.tile([1, T], F32, tag="row")
                nc.scalar.activation(out=mean, in_=s1, func=AF.Copy, scale=1.0 / D)
                ex2 = scr4.tile([1, T], F32, tag="row")
                nc.scalar.activation(out=ex2, in_=s2, func=AF.Copy, scale=1.0 / D)
                var = scr4.tile([1, T], F32, tag="row")
                nc.vector.tensor_mul(out=var, in0=mean, in1=mean)
                nc.vector.tensor_tensor(out=var, in0=ex2, in1=var,
                                        op=mybir.AluOpType.subtract)
                nc.scalar.activation(out=var, in_=var, func=AF.Ln, bias=eps_row[0:1, 0:1])
                rstd = scr4.tile([1, T], F32, tag="row")
                nc.scalar.activation(out=rstd, in_=var, func=AF.Exp, scale=-0.5)
                mb = lnps2.tile([128, T], F32, tag="mr")
                nc.tensor.matmul(mb, onesPP[0:1, :], mean, start=True, stop=True)
                rb = lnps2.tile([128, T], F32, tag="mr")
                nc.tensor.matmul(rb, onesPP[0:1, :], rstd, start=True, stop=True)
                for k in range(C):
                    d_t = scr2.tile([128, T], F32, tag="lnt")
                    nc.vector.tensor_tensor(out=d_t, in0=h[:, k, :], in1=mb,
                                            op=mybir.AluOpType.subtract)
                    nc.vector.tensor_tensor(out=d_t, in0=d_t, in1=rb,
                                            op=mybir.AluOpType.mult)
                    nc.vector.tensor_scalar(out=h[:, k, :], in0=d_t,
                                            scalar1=g_ap[:, k:k + 1],
                                            scalar2=b_ap[:, k:k + 1],
                                            op0=mybir.AluOpType.mult,
                                            op1=mybir.AluOpType.add)

        emit_ln(eg_s[:, :], eb_s[:, :])

        # ---------------- layers ----------------
        for l in range(n_layers):
            agi, ago = agis[l], agos[l]

            # bf16 cast of h for matmul inputs
            hb = sgl.tile([128, C, T], BF16, tag="hb")
            for k in range(C):
                nc.scalar.activation(out=hb[:, k, :], in_=h[:, k, :], func=AF.Copy)

            qT = sgl.tile([128, C, T], BF16, tag="qT")
            kT = sgl.tile([128, C, T], BF16, tag="kT")
            v65o = sgl.tile([128, 4, H, 65], BF16, tag="v65o")
            nc.vector.memset(v65o[:, :, :, 64:65], 1.0)

            with tc.tile_pool(name=f"psqkv{l}", bufs=3, space="PSUM") as psq:
                wq_t, wk_t, wv_t = [], [], []
                for k in range(C):
                    wt_ = wp4.tile([128, D], BF16, tag="w4")
                    nc.sync.dma_start(out=wt_, in_=wq[l, 128 * k:128 * (k + 1), :])
                    wq_t.append(wt_)
                for m in range(C):
                    ps = psq.tile([128, T], F32, tag="ps")
                    for k in range(C):
                        nc.tensor.matmul(ps, wq_t[k][:, 128 * m:128 * (m + 1)],
                                         hb[:, k, :], start=(k == 0), stop=(k == C - 1))
                    nc.vector.tensor_scalar_add(qT[:, m, :], ps, qb_s[:, l, m:m + 1])
                for k in range(C):
                    wt_ = wp4.tile([128, D], BF16, tag="w4")
                    nc.sync.dma_start(out=wt_, in_=wk[l, 128 * k:128 * (k + 1), :])
                    wk_t.append(wt_)
                for m in range(C):
                    ps = psq.tile([128, T], F32, tag="ps")
                    for k in range(C):
                        nc.tensor.matmul(ps, wk_t[k][:, 128 * m:128 * (m + 1)],
                                         hb[:, k, :], start=(k == 0), stop=(k == C - 1))
                    nc.vector.tensor_copy(out=kT[:, m, :], in_=ps)
                for k in range(C):
                    wt_ = wp4.tile([128, D], BF16, tag="w4")
                    nc.sync.dma_start(out=wt_, in_=wv[l, 128 * k:128 * (k + 1), :])
                    wv_t.append(wt_)
                for t in range(4):
                    ps = psq.tile([128, T], F32, tag="ps")
                    ps2 = psq.tile([128, 256], F32, tag="ps2")
                    for k in range(C):
                        nc.tensor.matmul(ps, hb[:, k, 128 * t:128 * (t + 1)],
                                         wv_t[k][:, 0:512], start=(k == 0), stop=(k == C - 1))
                        nc.tensor.matmul(ps2, hb[:, k, 128 * t:128 * (t + 1)],
                                         wv_t[k][:, 512:768], start=(k == 0), stop=(k == C - 1))
                    nc.vector.tensor_copy(
                        out=v65o[:, t, 0:8, 0:64],
                        in_=ps[:, :].rearrange("p (h d) -> p h d", h=8))
                    nc.vector.tensor_copy(
                        out=v65o[:, t, 8:12, 0:64],
                        in_=ps2[:, :].rearrange("p (h d) -> p h d", h=4))

            # ---- AllGather of (kT, V) within the 4-core group ----
            wkt = nc.sync.dma_start(
                out=agi[0:AGW].rearrange("(k p n) -> p k n", k=C, p=128, n=T),
                in_=kT[:, :, :])
            wvts = []
            for t in range(4):
                wvt = nc.sync.dma_start(
                    out=agi[AGW + t * 128 * D: AGW + (t + 1) * 128 * D].rearrange(
                        "(p h d) -> p h d", p=128, h=H, d=64),
                    in_=v65o[:, t, :, 0:64])
                wvts.append(wvt)
            if no_collective == "nohalo":
                cc = nc.gpsimd.collective_compute(
                    "AllGather", mybir.AluOpType.bypass, replica_groups=GROUPS,
                    ins=[agi[:]], outs=[ago[:, :]])
                add_dep_helper(cc.ins, wkt.ins, reason="AG waits kT bounce")
                for wvt in wvts:
                    add_dep_helper(cc.ins, wvt.ins, reason="AG waits V bounce")
            elif no_collective == "nodep":
                cc = nc.gpsimd.collective_compute(
                    "AllGather", mybir.AluOpType.bypass, replica_groups=GROUPS,
                    ins=[agi[:]], outs=[ago[:, :]])
                add_dep_helper(cc.ins, wkt.ins, reason="AG waits kT bounce")
                for wvt in wvts:
                    add_dep_helper(cc.ins, wvt.ins, reason="AG waits V bounce")
            elif no_collective == "selfgroup":
                cc = nc.gpsimd.collective_compute(
                    "AllGather", mybir.AluOpType.bypass,
                    replica_groups=[[c] for c in range(N_CORES)],
                    ins=[agi[:]], outs=[ago[0, :]])
                for rr in range(1, 4):
                    ccx = nc.sync.dma_start(out=ago[rr, :], in_=agi[:])
                    add_dep_helper(ccx.ins, wkt.ins, reason="bounce copy waits kT")
                    for wvt in wvts:
                        add_dep_helper(ccx.ins, wvt.ins, reason="bounce copy waits V")
            elif no_collective:
                cc = nc.sync.dma_start(out=ago[0, :], in_=agi[:])
                for rr in range(1, 4):
                    ccx = nc.sync.dma_start(out=ago[rr, :], in_=agi[:])
                    add_dep_helper(ccx.ins, wkt.ins, reason="bounce copy waits kT")
                    for wvt in wvts:
                        add_dep_helper(ccx.ins, wvt.ins, reason="bounce copy waits V")
            else:
                cc = nc.gpsimd.collective_compute(
                    "AllGather", mybir.AluOpType.bypass, replica_groups=GROUPS,
                    ins=[agi[:]], outs=[ago[:, :]])
            add_dep_helper(cc.ins, wkt.ins, reason="AG waits kT bounce")
            for wvt in wvts:
                add_dep_helper(cc.ins, wvt.ins, reason="AG waits V bounce")

            kTf = big.tile([128, C, 4, T], BF16, tag="kTf")
            v65f = big.tile([128, G, H, 65], BF16, tag="v65f")
            nc.vector.memset(v65f[:, :, :, 64:65], 1.0)
            for r in range(4):
                f1 = nc.sync.dma_start(
                    out=kTf[:, :, r, :],
                    in_=ago[r, 0:AGW].rearrange("(k p n) -> p k n", k=C, p=128, n=T))
                if no_collective != "nodep":
                    add_dep_helper(f1.ins, cc.ins, reason="kTf fetch waits AG")
                vsh = scr2.tile([128, 4, D], BF16, tag="vsh", name=f"vsh_{l}_{r}")
                f2 = nc.sync.dma_start(
                    out=vsh,
                    in_=ago[r, AGW:2 * AGW].rearrange("(t p d) -> p t d", t=4, p=128, d=D))
                if no_collective != "nodep":
                    add_dep_helper(f2.ins, cc.ins, reason="V fetch waits AG")
                for t in range(4):
                    g = 2 * r + t if t < 2 else 8 + 2 * r + (t - 2)
                    nc.vector.tensor_copy(
                        out=v65f[:, g, :, 0:64],
                        in_=vsh[:, t, :].rearrange("p (h d) -> p h d", h=H))

            if no_collective == "nohalo":
                kThalo = kT.rearrange("p k (s n) -> p k s n", s=4)[:, :, 0:2, :]
                v65h = v65o[:, 0:2, :, :]
            else:
                kThalo = sgl.tile([128, C, 2, 128], BF16, tag="kThalo")
                v65h = sgl.tile([128, 2, H, 65], BF16, tag="v65h")
                nc.vector.memset(v65h[:, :, :, 64:65], 1.0)
            for s, (rk, rv) in ([] if no_collective == "nohalo" else list(enumerate([(regKA, regVA), (regKB, regVB)]))):
                fh = nc.gpsimd.dma_start(
                    out=kThalo[:, :, s, :],
                    in_=bass.AP(ago, rk, [[T, 128], [128 * T, C], [1, 128]]))
                if no_collective != "nodep":
                    add_dep_helper(fh.ins, cc.ins, reason="kT halo waits AG")
                fv = nc.gpsimd.dma_start(
                    out=v65h[:, s, :, 0:64],
                    in_=bass.AP(ago, rv, [[D, 128], [64, H], [1, 64]]))
                if no_collective != "nodep":
                    add_dep_helper(fv.ins, cc.ins, reason="V halo waits AG")

            # ---- banded attention ----
            cb_sb = big.tile([128, H, T], BF16, tag="cbs")
            with tc.tile_pool(name=f"psA{l}", bufs=2, space="PSUM") as psa, \
                 tc.tile_pool(name=f"psAc{l}", bufs=2, space="PSUM") as psac:
                for hd_i in range(H):
                    hp, ck = 64 * (hd_i % 2), hd_i // 2
                    sc = psa.tile([128, 1024], F32, tag="scB")
                    segs = [
                        (0, 128, kThalo[hp:hp + 64, ck, 0, :], 0),
                        (128, 256, kT[hp:hp + 64, ck, 0:128], 0),
                        (384, 128, kT[hp:hp + 64, ck, 128:256], 128),
                        (512, 128, kThalo[hp:hp + 64, ck, 1, :], 256),
                        (640, 256, kT[hp:hp + 64, ck, 256:384], 256),
                        (896, 128, kT[hp:hp + 64, ck, 384:512], 384),
                    ]
                    bank_started = set()
                    for i, (cs, w, lhs, qs) in enumerate(segs):
                        bank = cs // 512
                        st = bank not in bank_started
                        bank_started.add(bank)
                        nc.tensor.matmul(sc[:, cs:cs + w], lhs,
                                         qT[hp:hp + 64, ck, qs:qs + w],
                                         start=st, stop=(i == len(segs) - 1),
                                         skip_group_check=True)
                    eb_t = epB.tile([128, 1024], BF16, tag="expB")
                    nc.scalar.activation(out=eb_t, in_=sc, func=AF.Exp, scale=SCALE)
                    nc.vector.tensor_mul(out=eb_t, in0=eb_t, in1=mband)
                    # AV: ctx psum rows 0:64 = ctx, row 64 = sumexp
                    cps = psac.tile([128, T], F32, tag="ctx")
                    av = [
                        (0, v65h[:, 0, hd_i, :], eb_t[:, 0:128]),
                        (0, v65o[:, 0, hd_i, :], eb_t[:, 128:256]),
                        (128, v65o[:, 0, hd_i, :], eb_t[:, 256:384]),
                        (128, v65o[:, 1, hd_i, :], eb_t[:, 384:512]),
                        (256, v65h[:, 1, hd_i, :], eb_t[:, 512:640]),
                        (256, v65o[:, 2, hd_i, :], eb_t[:, 640:768]),
                        (384, v65o[:, 2, hd_i, :], eb_t[:, 768:896]),
                        (384, v65o[:, 3, hd_i, :], eb_t[:, 896:1024]),
                    ]
                    for i, (ocs, vsrc, esrc) in enumerate(av):
                        nc.tensor.matmul(cps[0:65, ocs:ocs + 128], vsrc, esrc,
                                         start=(i == 0), stop=(i == len(av) - 1),
                                         skip_group_check=True)
                    nc.vector.tensor_copy(out=cb_sb[0:65, hd_i, :], in_=cps[0:65, :])

            # ---- dense attention for the B half (cols 256:512) ----
            with tc.tile_pool(name=f"psD{l}", bufs=3, space="PSUM") as psd, \
                 tc.tile_pool(name=f"psDc{l}", bufs=2, space="PSUM") as psdc:
                for hd_i in range(H):
                    hp, ck = 64 * (hd_i % 2), hd_i // 2
                    eds = []
                    for half in range(4):
                        sd = psd.tile([128, 1024], F32, tag="scD")
                        for i, g in enumerate(range(4 * half, 4 * half + 4)):
                            r, j = _owner(g), _lb(g)
                            nc.tensor.matmul(
                                sd[:, 256 * i:256 * (i + 1)],
                                kTf[hp:hp + 64, ck, r, 128 * j:128 * (j + 1)],
                                qT[hp:hp + 64, ck, 256:512],
                                start=(i % 2 == 0), stop=(i % 2 == 1),
                                skip_group_check=True)
                        ed = epD.tile([128, 1024], BF16, tag="expD")
                        nc.scalar.activation(out=ed, in_=sd, func=AF.Exp, scale=SCALE)
                        eds.append(ed)
                    cdps = psdc.tile([128, 256], F32, tag="ctxd")
                    for g in range(G):
                        nc.tensor.matmul(
                            cdps[0:65, :], v65f[:, g, hd_i, :],
                            eds[g // 4][:, 256 * (g % 4):256 * (g % 4 + 1)],
                            start=(g == 0), stop=(g == G - 1),
                            skip_group_check=True)
                    # blend: cb += m * (cd - cb)   (m is 0/1)
                    bl = scr2.tile([128, 256], F32, tag="bl")
                    nc.vector.tensor_tensor(out=bl[0:65, :], in0=cdps[0:65, :],
                                            in1=cb_sb[0:65, hd_i, 256:512],
                                            op=mybir.AluOpType.subtract)
                    nc.vector.tensor_tensor(out=bl[0:65, :], in0=bl[0:65, :],
                                            in1=msel[0:65, :],
                                            op=mybir.AluOpType.mult)
                    nc.vector.tensor_tensor(out=cb_sb[0:65, hd_i, 256:512],
                                            in0=cb_sb[0:65, hd_i, 256:512],
                                            in1=bl[0:65, :],
                                            op=mybir.AluOpType.add)

            # ---- normalize ctx, pack to feature-major ctxT ----
            ctxT = sgl.tile([128, C, T], BF16, tag="ctxT")
            with tc.tile_pool(name=f"psN{l}", bufs=2, space="PSUM") as psn:
                for c in range(C):
                    he, ho = 2 * c, 2 * c + 1
                    re_ = scr2.tile([128, T], F32, tag="rrow")
                    ro_ = scr2.tile([128, T], F32, tag="rrow")
                    nc.vector.reciprocal(out=re_[64:65, :], in_=cb_sb[64:65, he, :])
                    nc.vector.reciprocal(out=ro_[64:65, :], in_=cb_sb[64:65, ho, :])
                    rbe = psn.tile([128, T], F32, tag="rps")
                    nc.tensor.matmul(rbe[0:64, :], onesPP[64:65, 0:64], re_[64:65, :],
                                     start=True, stop=True)
                    rbo = psn.tile([128, T], F32, tag="rps")
                    nc.tensor.matmul(rbo[0:64, :], onesPP[64:65, 0:64], ro_[64:65, :],
                                     start=True, stop=True)
                    nc.vector.tensor_mul(out=ctxT[0:64, c, :],
                                         in0=cb_sb[0:64, he, :], in1=rbe[0:64, :])
                    nc.vector.tensor_mul(out=ctxT[64:128, c, :],
                                         in0=cb_sb[0:64, ho, :], in1=rbo[0:64, :])

            # ---- O projection + residual + LN ----
            with tc.tile_pool(name=f"psO{l}", bufs=2, space="PSUM") as pso:
                wo_t = []
                for k in range(C):
                    wt_ = wp4.tile([128, D], BF16, tag="w4")
                    nc.sync.dma_start(out=wt_, in_=wo[l, 128 * k:128 * (k + 1), :])
                    wo_t.append(wt_)
                for m in range(C):
                    ps = pso.tile([128, T], F32, tag="ops")
                    for k in range(C):
                        nc.tensor.matmul(ps, wo_t[k][:, 128 * m:128 * (m + 1)],
                                         ctxT[:, k, :], start=(k == 0), stop=(k == C - 1))
                    t_ = scr2.tile([128, T], F32, tag="res")
                    nc.vector.tensor_scalar_add(t_, ps, ob_s[:, l, m:m + 1])
                    nc.vector.tensor_add(out=h[:, m, :], in0=h[:, m, :], in1=t_)
            emit_ln(lng_s[:, l, 0, :], lnb_s[:, l, 0, :])

            # ---- FFN + residual + LN ----
            hb2 = sgl.tile([128, C, T], BF16, tag="hb")
            for k in range(C):
                nc.scalar.activation(out=hb2[:, k, :], in_=h[:, k, :], func=AF.Copy)
            with tc.tile_pool(name=f"psF1{l}", bufs=2, space="PSUM") as psf1, \
                 tc.tile_pool(name=f"psF2{l}", bufs=6, space="PSUM") as psf2:
                f2ps = [psf2.tile([128, T], F32, tag="f2", name=f"f2ps_{l}_{m}") for m in range(C)]
                for half in range(2):
                    w1_t = []
                    for k in range(C):
                        wt_ = wp1.tile([128, FF // 2], BF16, tag="w1t")
                        nc.sync.dma_start(
                            out=wt_, in_=w1[l, 128 * k:128 * (k + 1),
                                            half * (FF // 2):(half + 1) * (FF // 2)])
                        w1_t.append(wt_)
                    for fi in range(FFC // 2):
                        f = half * (FFC // 2) + fi
                        w2_t = wp2.tile([128, D], BF16, tag="w2t")
                        nc.sync.dma_start(out=w2_t, in_=w2[l, 128 * f:128 * (f + 1), :])
                        ps1 = psf1.tile([128, T], F32, tag="f1")
                        for k in range(C):
                            nc.tensor.matmul(ps1, w1_t[k][:, 128 * fi:128 * (fi + 1)],
                                             hb2[:, k, :], start=(k == 0), stop=(k == C - 1))
                        rl = scr3.tile([128, T], BF16, tag="rl")
                        nc.scalar.activation(out=rl, in_=ps1, func=AF.Relu,
                                             bias=f1b_s[:, l, f:f + 1])
                        for m in range(C):
                            nc.tensor.matmul(f2ps[m], w2_t[:, 128 * m:128 * (m + 1)], rl,
                                             start=(f == 0), stop=(f == FFC - 1))
                for m in range(C):
                    t_ = scr2.tile([128, T], F32, tag="res")
                    nc.vector.tensor_scalar_add(t_, f2ps[m], f2b_s[:, l, m:m + 1])
                    nc.vector.tensor_add(out=h[:, m, :], in0=h[:, m, :], in1=t_)
            emit_ln(lng_s[:, l, 1, :], lnb_s[:, l, 1, :])

        nc.sync.dma_start(out=out_d[:, :, :], in_=h)
    return nc


def _feat_pack(v):
    """[768] -> [128, 6] feature-major"""
    return np.ascontiguousarray(np.asarray(v, np.float32).reshape(C, 128).T)


def _tok_pack(arr):
    """[512, 768] -> [128, 6, 512] feature-major"""
    a = np.asarray(arr, np.float32).T.reshape(C, 128, T)
    return np.ascontiguousarray(a.transpose(1, 0, 2))


def make_in_maps(inputs):
    lengths = np.asarray(inputs["lengths"]).astype(np.int64)
    window = int(np.asarray(inputs["window"]))
    assert window == WINDOW and np.all(lengths >= S // 2)
    x = np.asarray(inputs["inputs"], np.float32)
    pos = np.asarray(inputs["pos_emb"], np.float32)
    tok = np.asarray(inputs["tok_emb"], np.float32)

    bf = ml_dtypes.bfloat16
    ow = np.asarray(inputs["o_w"], np.float32)
    vb = np.asarray(inputs["v_b"], np.float32)
    ob_eff = np.asarray(inputs["o_b"], np.float32) + np.einsum("ld,lde->le", vb, ow)

    shared = {
        "tokf": _feat_pack(tok[0]),
        "eg": _feat_pack(inputs["emb_ln_g"]),
        "ebi": _feat_pack(inputs["emb_ln_b"]),
        "lng": np.ascontiguousarray(np.stack(
            [np.stack([_feat_pack(np.asarray(inputs["attn_ln_g"])[li]),
                       _feat_pack(np.asarray(inputs["out_ln_g"])[li])], 1)
             for li in range(L)], 1), np.float32),
        "lnb": np.ascontiguousarray(np.stack(
            [np.stack([_feat_pack(np.asarray(inputs["attn_ln_b"])[li]),
                       _feat_pack(np.asarray(inputs["out_ln_b"])[li])], 1)
             for li in range(L)], 1), np.float32),
        "qbi": np.ascontiguousarray(np.stack(
            [_feat_pack(np.asarray(inputs["q_b"])[li]) for li in range(L)], 1), np.float32),
        "obi": np.ascontiguousarray(np.stack(
            [_feat_pack(ob_eff[li]) for li in range(L)], 1), np.float32),
        "f1bi": np.ascontiguousarray(np.stack(
            [np.ascontiguousarray(np.asarray(inputs["ff1_b"], np.float32)[li].reshape(FFC, 128).T)
             for li in range(L)], 1), np.float32),
        "f2bi": np.ascontiguousarray(np.stack(
            [_feat_pack(np.asarray(inputs["ff2_b"])[li]) for li in range(L)], 1), np.float32),
        "wq": np.asarray(inputs["q_w"], np.float32).astype(bf),
        "wk": np.asarray(inputs["k_w"], np.float32).astype(bf),
        "wv": np.asarray(inputs["v_w"], np.float32).astype(bf),
        "wo": np.asarray(inputs["o_w"], np.float32).astype(bf),
        "w1": np.asarray(inputs["ff1_w"], np.float32).astype(bf),
        "w2": np.asarray(inputs["ff2_w"], np.float32).astype(bf),
    }

    io = np.arange(128)
    m_own = (io[:, None] <= io[None, :]).astype(np.float32)      # key off <= q off
    m_prev = (io[:, None] > io[None, :]).astype(np.float32)      # prev-block band
    in_maps = []
    for cidx in range(N_CORES):
        b, q = cidx // 4, cidx % 4
        rows = _rows_for(q)
        lb_ = int(lengths[b])
        mhA = np.zeros((128, 128), np.float32) if q == 0 else m_prev
        # segments: [haloA | b0:own+prev | b1:own | haloB:prev | b2:own+prev | b3:own]
        mb_ = np.concatenate([mhA, m_own, m_prev, m_own,
                              m_prev, m_own, m_prev, m_own], axis=1)
        msel_ = (np.asarray([1024 + 256 * q + cc_ for cc_ in range(256)]) >= lb_)
        gA = 2 * q - 1 if q > 0 else 15
        gB = 7 + 2 * q
        hoff = []
        for g in (gA, gB):
            hoff.append(_owner(g) * 2 * AGW + 128 * _lb(g))
        for g in (gA, gB):
            hoff.append(_owner(g) * 2 * AGW + AGW + 128 * _lb(g) * D)
        im = dict(shared)
        im["xT"] = _tok_pack(x[b][rows])
        im["posT"] = _tok_pack(pos[rows])
        im["mbandi"] = mb_.astype(bf)
        im["mseli"] = np.broadcast_to(msel_, (128, 256)).astype(bf).copy()
        im["hoffi"] = np.asarray([hoff], np.int32)
        in_maps.append(im)
    return in_maps


def assemble_output(results):
    out = np.zeros((B, S, D), np.float32)
    for cidx in range(N_CORES):
        b, q = cidx // 4, cidx % 4
        rows = _rows_for(q)
        hc = np.asarray(results[cidx]["out"])          # [128, 6, 512]
        out[b, rows, :] = hc.transpose(1, 0, 2).reshape(D, T).T
    return out


_cached = {}


def kernel(**inputs):
    n_layers = int(inputs.pop("_n_layers", L))
    if n_layers not in _cached:
        _cached[n_layers] = build_nc(n_layers)
    nc = _cached[n_layers]
    in_maps = make_in_maps(inputs)
    res = run_bass_kernel_spmd(nc, in_maps, core_ids=list(range(N_CORES)))
    return assemble_output(res.results)

